# revision 28
# baseline (speedup 1.0000x reference)
"""GQA cross-attention kernel for Trainium2 (8 NeuronCores, Bass/Tile).

Problem: q (2,2048,16,64) f32, kv (2,2048,2,4,64) f32, key_padding_mask (2,2048)
bool.  Reference: GQA attention with additive -10000 padding bias and a causal
mask shifted by the per-batch valid key count sk, softmax over keys.

Key observations used here:
  * Every padded key position is also causal-masked (the where() sets those
    scores to exactly -10000), so only the shifted-causal structure matters.
  * With u := q_idx - c (c = 2048 - sk), the valid region is exactly u >= k,
    a standard causal triangle, and only keys k < sk participate.  The shift
    is applied on the HOST when laying out Q^T per core, so the device
    program is a static causal flash-attention kernel.
  * Rows q_idx < c have no valid key: the reference softmaxes a row of equal
    -10000s -> uniform weights -> output = mean over ALL 2048 v rows.  Pure
    host-side fixup.
  * exp without max-subtraction is safe (|score*0.125| <~ 8), and the softmax
    denominator is obtained by appending a ones-column to V (PV matmul then
    yields [num | den]); the division happens on host.

Device program (per core, 4 head-instances = 2 heads x 2 batches, mixed
batch sharding so every core gets an identical causal workload):
  S^T[k,u] = K^T.T @ Q^T   (bf16 matmuls; D zero-padded 64->128 so every
                            matmul runs K=128: avoids PE row-group switches)
  P^T      = exp(0.125 * S^T)        (ScalarE, PSUM -> SBUF, per-row tiles)
  diagonal 128x128 blocks masked by a host-provided triangle (VectorE mul)
  [num|den]^T += V'(k-tile).T @ P^T  (bf16, chunk-serial PSUM accumulation,
                            drip-scheduled 2 rows behind exp so the in-order
                            PE stream never blocks and holds 2.4 GHz)
  PSUM -> SBUF copy (VectorE), DMA out^T [65, 2048] per instance.
"""

import os
import ml_dtypes
import numpy as np

BF16 = ml_dtypes.bfloat16

import concourse.bass as bass
import concourse.mybir as mybir
import concourse.tile as tile
from concourse import bacc
from concourse.bass_utils import run_bass_kernel_spmd

B, SQ, SK, H, HK, D = 2, 2048, 2048, 16, 4, 64
NCORES = 8
P = 128
FP = mybir.dt.float32
FR = mybir.dt.bfloat16
S_TILE = 1024  # width of one PSUM scores strip (2 banks)
ACC_W = 512    # width of one PV accumulator chunk (1 bank)

LAST_EXEC_NS = None


def _ceil_div(a, b):
    return -(-a // b)


def _build_program(sks):
    """Build + compile the SPMD program for per-batch valid key counts sks.

    Schedule: per key-tile row kt, QK strips ([128, <=1536] PSUM, 3 banks,
    double-buffered) -> one exp ACT per strip into a per-row P^T tile that
    persists in SBUF -> diagonal tri-mask on DVE.  PV runs chunk-serially:
    once all rows a 512-wide output chunk needs are exp'd, a burst of PV
    matmuls accumulates that chunk in one of two rotating PSUM accumulators.
    This keeps the PE instruction stream dependency-clean (it ramps to
    2.4 GHz only when never blocked) and minimizes Scalar ACT count (the
    exp engine is the pacer at ~0.83 ns/col + ~150ns per instruction).
    """
    nc = bacc.Bacc("TRN2", target_bir_lowering=False, debug=False,
                   num_devices=NCORES)

    # D=64 zero-padded to 128 contraction rows: every matmul then runs with
    # K=128, avoiding PE row-group reconfiguration between QK (else K=64)
    # and PV (K=128) instructions, which costs ~0.2-0.4 ns/col in practice
    qT_d = nc.dram_tensor("qT", [4, P, SQ], FR, kind="ExternalInput").ap()
    kT_d = nc.dram_tensor("kT", [B, P, SK], FR, kind="ExternalInput").ap()
    vp_d = nc.dram_tensor("vp", [B, P, (SK // P) * 65], FR,
                          kind="ExternalInput").ap()
    tri_d = nc.dram_tensor("tri", [P, P], FR, kind="ExternalInput").ap()
    out_d = nc.dram_tensor("outT", [4, 65, SQ], FP, kind="ExternalOutput").ap()

    EXP = mybir.ActivationFunctionType.Exp
    S_W = 1536  # QK strip width: 3 PSUM banks

    with tile.TileContext(nc) as tc:
        with (
            tc.tile_pool(name="const", bufs=1) as cpool,
            tc.tile_pool(name="kv", bufs=1) as kvpool,
            tc.tile_pool(name="qin", bufs=1) as qpool,
            tc.tile_pool(name="pt", bufs=2) as ppool,
            tc.tile_pool(name="oc", bufs=2) as opool,
            tc.tile_pool(name="ps", bufs=2, space="PSUM") as spool,
            tc.tile_pool(name="pa", bufs=2, space="PSUM") as apool,
        ):
            kT_sb = []
            vp_sb = []
            for b in range(B):
                kt_t = kvpool.tile([P, SK], FR, name=f"kT{b}", tag=f"kT{b}")
                kT_sb.append(kt_t)
                vp_t = kvpool.tile([P, (SK // P) * 65], FR, name=f"vp{b}",
                                   tag=f"vp{b}")
                vp_sb.append(vp_t)
            tri_sb = cpool.tile([P, P], FR, name="tri_sb")
            q_tiles = [qpool.tile([P, SQ], FR, name=f"q{j}", tag=f"q{j}")
                       for j in range(4)]
            # preloads ordered so the first QK matmul (needs kT0[:, :128] and
            # q0[:, :512]) unblocks after ~2 transfers; DMA issue on the sync
            # queue is serial (~0.7us each) so keep the count low
            nc.sync.dma_start(kT_sb[0][:, 0:128], kT_d[0][:, 0:128])
            nc.sync.dma_start(q_tiles[0][:, 0:512], qT_d[0][:, 0:512])
            nc.sync.dma_start(kT_sb[0][:, 128:512], kT_d[0][:, 128:512])
            nc.sync.dma_start(q_tiles[0][:, 512:1024], qT_d[0][:, 512:1024])
            nc.sync.dma_start(q_tiles[0][:, 1024:2048], qT_d[0][:, 1024:2048])
            nc.sync.dma_start(tri_sb[:], tri_d[:])
            nc.sync.dma_start(vp_sb[0][:], vp_d[0])
            nc.sync.dma_start(kT_sb[0][:, 512:2048], kT_d[0][:, 512:2048])
            nc.sync.dma_start(q_tiles[1][:], qT_d[1])
            nc.sync.dma_start(kT_sb[1][:], kT_d[1])
            nc.sync.dma_start(vp_sb[1][:], vp_d[1])
            nc.sync.dma_start(q_tiles[2][:], qT_d[2])
            nc.sync.dma_start(q_tiles[3][:], qT_d[3])

            def build_instance(j, prev_flush):
                """Emit one head-instance; returns its tail-flush closure.

                prev_flush (the previous instance's leftover PV items + final
                out-DMA) is emitted after this instance's first two QK rows:
                those items depend on the previous instance's last exps, so
                deferring them keeps the in-order PE stream unblocked at the
                instance boundary.
                """
                b = 0 if j < 2 else 1
                U = sks[b]
                KT = _ceil_div(U, P)
                NCH = _ceil_div(U, ACC_W)

                q_sb = q_tiles[j]
                pt_rows = [ppool.tile([P, SQ], FR, name=f"pt{kt}",
                                      tag=f"pt{kt}") for kt in range(KT)]
                oc = opool.tile([65, SQ], FP, name="oc", tag="oc")

                def chunk_last(c):
                    return min(KT - 1, ((c + 1) * ACC_W - 1) // P)

                # PV work list, chunk-major (accs rotate through 2 PSUM
                # buffers, so chunks must open/close in order)
                pv_items = [(c, kt2) for c in range(NCH)
                            for kt2 in range(chunk_last(c) + 1)]
                pv_state = {"i": 0, "acc": None}

                def emit_pv_item(c, kt2):
                    last = chunk_last(c)
                    a0c = c * ACC_W
                    a1 = min(U, (c + 1) * ACC_W)
                    if kt2 == 0:
                        pv_state["acc"] = apool.tile([65, ACC_W], FP,
                                                     name="acc", tag="acc")
                    acc = pv_state["acc"]
                    a0 = max(P * kt2, a0c)
                    # full 128 contraction rows even on the tail key-tile:
                    # its invalid P^T rows are zeroed once, so the (real,
                    # finite) vp values there contribute exactly zero
                    nc.tensor.matmul(
                        acc[:, a0 - a0c:a1 - a0c],
                        lhsT=vp_sb[b][:, 65 * kt2:65 * (kt2 + 1)],
                        rhs=pt_rows[kt2][:, a0:a1],
                        start=(kt2 == 0), stop=(kt2 == last),
                        skip_group_check=True,
                    )
                    if kt2 == last:
                        nc.vector.tensor_copy(oc[:, a0c:a1],
                                              acc[:, 0:a1 - a0c])

                def drip_pv(row):
                    # emit PV items at least two rows behind QK: when the PE
                    # reaches QK(row) the strip buffer rotation (bufs=2)
                    # guarantees exp(row-2) has completed, so these items can
                    # never block the in-order PE stream
                    while pv_state["i"] < len(pv_items):
                        c, kt2 = pv_items[pv_state["i"]]
                        if kt2 > row - 2:
                            break
                        emit_pv_item(c, kt2)
                        pv_state["i"] += 1

                for kt in range(KT):
                    u0 = P * kt
                    kw = min(P, U - u0)
                    if kw < P:
                        # zero the tail key-tile's invalid P^T rows so PV can
                        # use the full 128 contraction rows; runs before exp,
                        # which then overwrites the valid rows below kw.
                        # (engine partition base must be 32-aligned)
                        st = (kw // 32) * 32
                        nc.vector.memset(pt_rows[kt][st:P, u0:U], 0.0)
                    s0 = u0
                    while s0 < U:
                        s1 = min(U, s0 + S_W)
                        ps = spool.tile([P, S_W], FP, name="ps", tag="ps")
                        # <=512-wide matmul pieces: each output must stay
                        # inside one PSUM bank (bank = 512 fp32 cols)
                        m0 = s0
                        while m0 < s1:
                            m1 = min(s1, m0 + 512)
                            nc.tensor.matmul(
                                ps[0:kw, m0 - s0:m1 - s0],
                                lhsT=kT_sb[b][:, u0:u0 + kw],
                                rhs=q_sb[:, m0:m1],
                                start=True, stop=True,
                                skip_group_check=True,
                            )
                            m0 = m1
                        nc.scalar.activation(pt_rows[kt][0:kw, s0:s1],
                                             ps[0:kw, 0:s1 - s0],
                                             EXP, scale=0.125)
                        s0 = s1
                    dw = min(P, U - u0)
                    nc.vector.tensor_mul(pt_rows[kt][0:kw, u0:u0 + dw],
                                         pt_rows[kt][0:kw, u0:u0 + dw],
                                         tri_sb[0:kw, 0:dw])
                    drip_pv(kt)
                    if kt == 1 and prev_flush is not None:
                        prev_flush()
                    if kt == KT - 1 and U > 1152:
                        # first half of the output can ship while the tail
                        # chunks are still accumulating
                        nc.sync.dma_start(out_d[j, :, 0:1024], oc[:, 0:1024])

                def flush():
                    drip_pv(KT + 1)
                    if U > 1152:
                        nc.sync.dma_start(out_d[j, :, 1024:U], oc[:, 1024:U])
                    else:
                        nc.sync.dma_start(out_d[j, :, 0:U], oc[:, 0:U])
                return flush

            fl = None
            for j in range(4):
                fl = build_instance(j, fl)
            fl()

    nc.compile()
    return nc


_prog_cache = {}


def _get_program(sks):
    if sks not in _prog_cache:
        _prog_cache[sks] = _build_program(sks)
    return _prog_cache[sks]


def kernel(q, kv, key_padding_mask):
    global LAST_EXEC_NS
    q = np.asarray(q, dtype=np.float32)
    kv = np.asarray(kv, dtype=np.float32)
    mask = np.asarray(key_padding_mask)

    sk = mask.sum(axis=1).astype(np.int64)  # (B,) valid key counts
    c = (SQ - sk).astype(np.int64)
    prog = _get_program((int(sk[0]), int(sk[1])))

    k_all = kv[:, :, 0]  # (B, SK, HK, D)
    v_all = kv[:, :, 1]

    tri = (np.arange(P)[None, :] >= np.arange(P)[:, None]).astype(np.float32)

    kT_by_g = {}
    vp_by_g = {}
    for g in range(HK):
        kTg = np.zeros((B, P, SK), dtype=np.float32)
        kTg[:, :D, :] = k_all[:, :, g, :].transpose(0, 2, 1)
        kT_by_g[g] = kTg
        vpz = np.ones((B, SK, 65), dtype=np.float32)
        vpz[:, :, :64] = v_all[:, :, g, :]
        vp = vpz.reshape(B, SK // P, P, 65).transpose(0, 2, 1, 3)
        vp_by_g[g] = np.ascontiguousarray(vp.reshape(B, P, (SK // P) * 65))

    def core_instances(core):
        g = core // 2
        hp = core % 2
        h0 = 4 * g + 2 * hp
        return g, [(0, h0), (0, h0 + 1), (1, h0), (1, h0 + 1)]

    in_maps = []
    for core in range(NCORES):
        g, insts = core_instances(core)
        qT = np.zeros((4, P, SQ), dtype=np.float32)
        for jj, (b, h) in enumerate(insts):
            U = int(sk[b])
            qT[jj, :D, :U] = q[b, c[b]:, h, :].T
        in_maps.append({
            "qT": qT.astype(BF16),
            "kT": kT_by_g[g].astype(BF16),
            "vp": vp_by_g[g].astype(BF16),
            "tri": tri.astype(BF16),
        })

    trace = bool(os.environ.get("BASS_KERNEL_TRACE"))
    res = run_bass_kernel_spmd(prog, in_maps, list(range(NCORES)),
                               trace=trace)
    LAST_EXEC_NS = res.exec_time_ns

    out = np.empty((B, SQ, H, D), dtype=np.float32)
    # fully-masked rows: uniform softmax over all SK keys -> mean of v
    vmean = v_all.mean(axis=1)  # (B, HK, D)
    for b in range(B):
        if c[b] > 0:
            for g in range(HK):
                for h in range(4 * g, 4 * g + 4):
                    out[b, :c[b], h, :] = vmean[b, g]

    for core in range(NCORES):
        g, insts = core_instances(core)
        o = res.results[core]["outT"]  # (4, 65, SQ)
        for jj, (b, h) in enumerate(insts):
            U = int(sk[b])
            num = o[jj, :64, :U]
            den = o[jj, 64, :U]
            out[b, c[b]:, h, :] = (num / den[None, :]).T

    return out



# revision 30
# speedup vs baseline: 1.0023x; 1.0023x over previous
"""GQA cross-attention kernel for Trainium2 (8 NeuronCores, Bass/Tile).

Problem: q (2,2048,16,64) f32, kv (2,2048,2,4,64) f32, key_padding_mask (2,2048)
bool.  Reference: GQA attention with additive -10000 padding bias and a causal
mask shifted by the per-batch valid key count sk, softmax over keys.

Key observations used here:
  * Every padded key position is also causal-masked (the where() sets those
    scores to exactly -10000), so only the shifted-causal structure matters.
  * With u := q_idx - c (c = 2048 - sk), the valid region is exactly u >= k,
    a standard causal triangle, and only keys k < sk participate.  The shift
    is applied on the HOST when laying out Q^T per core, so the device
    program is a static causal flash-attention kernel.
  * Rows q_idx < c have no valid key: the reference softmaxes a row of equal
    -10000s -> uniform weights -> output = mean over ALL 2048 v rows.  Pure
    host-side fixup.
  * exp without max-subtraction is safe (|score*0.125| <~ 8), and the softmax
    denominator is obtained by appending a ones-column to V (PV matmul then
    yields [num | den]); the division happens on host.

Device program (per core, 4 head-instances = 2 heads x 2 batches, mixed
batch sharding so every core gets an identical causal workload):
  S^T[k,u] = K^T.T @ Q^T   (bf16 matmuls; D zero-padded 64->128 so every
                            matmul runs K=128: avoids PE row-group switches)
  P^T      = exp(0.125 * S^T)        (ScalarE, PSUM -> SBUF, per-row tiles)
  diagonal 128x128 blocks masked by a host-provided triangle (VectorE mul)
  [num|den]^T += V'(k-tile).T @ P^T  (bf16, chunk-serial PSUM accumulation,
                            drip-scheduled 2 rows behind exp so the in-order
                            PE stream never blocks and holds 2.4 GHz)
  PSUM -> SBUF copy (VectorE), DMA out^T [65, 2048] per instance.
"""

import os
import ml_dtypes
import numpy as np

BF16 = ml_dtypes.bfloat16

import concourse.bass as bass
import concourse.mybir as mybir
import concourse.tile as tile
from concourse import bacc
from concourse.bass_utils import run_bass_kernel_spmd

B, SQ, SK, H, HK, D = 2, 2048, 2048, 16, 4, 64
NCORES = 8
P = 128
FP = mybir.dt.float32
FR = mybir.dt.bfloat16
S_TILE = 1024  # width of one PSUM scores strip (2 banks)
ACC_W = 512    # width of one PV accumulator chunk (1 bank)

LAST_EXEC_NS = None


def _ceil_div(a, b):
    return -(-a // b)


def _build_program(sks):
    """Build + compile the SPMD program for per-batch valid key counts sks.

    Schedule: per key-tile row kt, QK strips ([128, <=1536] PSUM, 3 banks,
    double-buffered) -> one exp ACT per strip into a per-row P^T tile that
    persists in SBUF -> diagonal tri-mask on DVE.  PV runs chunk-serially:
    once all rows a 512-wide output chunk needs are exp'd, a burst of PV
    matmuls accumulates that chunk in one of two rotating PSUM accumulators.
    This keeps the PE instruction stream dependency-clean (it ramps to
    2.4 GHz only when never blocked) and minimizes Scalar ACT count (the
    exp engine is the pacer at ~0.83 ns/col + ~150ns per instruction).
    """
    nc = bacc.Bacc("TRN2", target_bir_lowering=False, debug=False,
                   num_devices=NCORES)

    # D=64 zero-padded to 128 contraction rows: every matmul then runs with
    # K=128, avoiding PE row-group reconfiguration between QK (else K=64)
    # and PV (K=128) instructions, which costs ~0.2-0.4 ns/col in practice
    qT_d = nc.dram_tensor("qT", [4, P, SQ], FR, kind="ExternalInput").ap()
    kT_d = nc.dram_tensor("kT", [B, P, SK], FR, kind="ExternalInput").ap()
    vp_d = nc.dram_tensor("vp", [B, P, (SK // P) * 65], FR,
                          kind="ExternalInput").ap()
    tri_d = nc.dram_tensor("tri", [P, P], FR, kind="ExternalInput").ap()
    out_d = nc.dram_tensor("outT", [4, 65, SQ], FP, kind="ExternalOutput").ap()

    EXP = mybir.ActivationFunctionType.Exp
    S_W = 1536  # QK strip width: 3 PSUM banks

    with tile.TileContext(nc) as tc:
        with (
            tc.tile_pool(name="const", bufs=1) as cpool,
            tc.tile_pool(name="kv", bufs=1) as kvpool,
            tc.tile_pool(name="qin", bufs=1) as qpool,
            tc.tile_pool(name="pt", bufs=2) as ppool,
            tc.tile_pool(name="oc", bufs=2) as opool,
            tc.tile_pool(name="ps", bufs=2, space="PSUM") as spool,
            tc.tile_pool(name="pa", bufs=2, space="PSUM") as apool,
        ):
            kT_sb = []
            vp_sb = []
            for b in range(B):
                kt_t = kvpool.tile([P, SK], FR, name=f"kT{b}", tag=f"kT{b}")
                kT_sb.append(kt_t)
                vp_t = kvpool.tile([P, (SK // P) * 65], FR, name=f"vp{b}",
                                   tag=f"vp{b}")
                vp_sb.append(vp_t)
            tri_sb = cpool.tile([P, P], FR, name="tri_sb")
            q_tiles = [qpool.tile([P, SQ], FR, name=f"q{j}", tag=f"q{j}")
                       for j in range(4)]
            # preloads ordered so the first QK matmul (needs kT0[:, :128] and
            # q0[:, :512]) unblocks after ~2 transfers; DMA issue on the sync
            # queue is serial (~0.7us each) so keep the count low
            nc.sync.dma_start(kT_sb[0][:, 0:128], kT_d[0][:, 0:128])
            nc.sync.dma_start(q_tiles[0][:, 0:512], qT_d[0][:, 0:512])
            nc.sync.dma_start(q_tiles[0][:, 512:1024], qT_d[0][:, 512:1024])
            nc.sync.dma_start(q_tiles[0][:, 1024:1536], qT_d[0][:, 1024:1536])
            nc.sync.dma_start(kT_sb[0][:, 128:512], kT_d[0][:, 128:512])
            nc.sync.dma_start(q_tiles[0][:, 1536:2048], qT_d[0][:, 1536:2048])
            nc.sync.dma_start(tri_sb[:], tri_d[:])
            nc.sync.dma_start(vp_sb[0][:], vp_d[0])
            nc.sync.dma_start(kT_sb[0][:, 512:2048], kT_d[0][:, 512:2048])
            nc.sync.dma_start(q_tiles[1][:], qT_d[1])
            nc.sync.dma_start(kT_sb[1][:], kT_d[1])
            nc.sync.dma_start(vp_sb[1][:], vp_d[1])
            nc.sync.dma_start(q_tiles[2][:], qT_d[2])
            nc.sync.dma_start(q_tiles[3][:], qT_d[3])

            def build_instance(j, prev_flush):
                """Emit one head-instance; returns its tail-flush closure.

                prev_flush (the previous instance's leftover PV items + final
                out-DMA) is emitted after this instance's first two QK rows:
                those items depend on the previous instance's last exps, so
                deferring them keeps the in-order PE stream unblocked at the
                instance boundary.
                """
                b = 0 if j < 2 else 1
                U = sks[b]
                KT = _ceil_div(U, P)
                NCH = _ceil_div(U, ACC_W)

                q_sb = q_tiles[j]
                pt_rows = [ppool.tile([P, SQ], FR, name=f"pt{kt}",
                                      tag=f"pt{kt}") for kt in range(KT)]
                oc = opool.tile([65, SQ], FP, name="oc", tag="oc")

                def chunk_last(c):
                    return min(KT - 1, ((c + 1) * ACC_W - 1) // P)

                # PV work list, chunk-major (accs rotate through 2 PSUM
                # buffers, so chunks must open/close in order)
                pv_items = [(c, kt2) for c in range(NCH)
                            for kt2 in range(chunk_last(c) + 1)]
                pv_state = {"i": 0, "acc": None}

                def emit_pv_item(c, kt2):
                    last = chunk_last(c)
                    a0c = c * ACC_W
                    a1 = min(U, (c + 1) * ACC_W)
                    if kt2 == 0:
                        pv_state["acc"] = apool.tile([65, ACC_W], FP,
                                                     name="acc", tag="acc")
                    acc = pv_state["acc"]
                    a0 = max(P * kt2, a0c)
                    # full 128 contraction rows even on the tail key-tile:
                    # its invalid P^T rows are zeroed once, so the (real,
                    # finite) vp values there contribute exactly zero
                    nc.tensor.matmul(
                        acc[:, a0 - a0c:a1 - a0c],
                        lhsT=vp_sb[b][:, 65 * kt2:65 * (kt2 + 1)],
                        rhs=pt_rows[kt2][:, a0:a1],
                        start=(kt2 == 0), stop=(kt2 == last),
                        skip_group_check=True,
                    )
                    if kt2 == last:
                        nc.vector.tensor_copy(oc[:, a0c:a1],
                                              acc[:, 0:a1 - a0c])

                def drip_pv(row):
                    # emit PV items at least two rows behind QK: when the PE
                    # reaches QK(row) the strip buffer rotation (bufs=2)
                    # guarantees exp(row-2) has completed, so these items can
                    # never block the in-order PE stream
                    while pv_state["i"] < len(pv_items):
                        c, kt2 = pv_items[pv_state["i"]]
                        if kt2 > row - 2:
                            break
                        emit_pv_item(c, kt2)
                        pv_state["i"] += 1

                for kt in range(KT):
                    u0 = P * kt
                    kw = min(P, U - u0)
                    if kw < P:
                        # zero the tail key-tile's invalid P^T rows so PV can
                        # use the full 128 contraction rows; runs before exp,
                        # which then overwrites the valid rows below kw.
                        # (engine partition base must be 32-aligned)
                        st = (kw // 32) * 32
                        nc.vector.memset(pt_rows[kt][st:P, u0:U], 0.0)
                    s0 = u0
                    while s0 < U:
                        # instance 0 row 0 leads with a 512-wide strip so the
                        # first exp fires after only 128KB of q has arrived
                        w_lim = 512 if (j == 0 and kt == 0 and s0 == u0) \
                            else S_W
                        s1 = min(U, s0 + w_lim)
                        ps = spool.tile([P, S_W], FP, name="ps", tag="ps")
                        # <=512-wide matmul pieces: each output must stay
                        # inside one PSUM bank (bank = 512 fp32 cols)
                        m0 = s0
                        while m0 < s1:
                            m1 = min(s1, m0 + 512)
                            nc.tensor.matmul(
                                ps[0:kw, m0 - s0:m1 - s0],
                                lhsT=kT_sb[b][:, u0:u0 + kw],
                                rhs=q_sb[:, m0:m1],
                                start=True, stop=True,
                                skip_group_check=True,
                            )
                            m0 = m1
                        nc.scalar.activation(pt_rows[kt][0:kw, s0:s1],
                                             ps[0:kw, 0:s1 - s0],
                                             EXP, scale=0.125)
                        s0 = s1
                    dw = min(P, U - u0)
                    nc.vector.tensor_mul(pt_rows[kt][0:kw, u0:u0 + dw],
                                         pt_rows[kt][0:kw, u0:u0 + dw],
                                         tri_sb[0:kw, 0:dw])
                    drip_pv(kt)
                    if kt == 1 and prev_flush is not None:
                        prev_flush()
                    if kt == KT - 1 and U > 1152:
                        # first half of the output can ship while the tail
                        # chunks are still accumulating
                        nc.sync.dma_start(out_d[j, :, 0:1024], oc[:, 0:1024])

                def flush():
                    drip_pv(KT + 1)
                    if U > 1152:
                        nc.sync.dma_start(out_d[j, :, 1024:U], oc[:, 1024:U])
                    else:
                        nc.sync.dma_start(out_d[j, :, 0:U], oc[:, 0:U])
                return flush

            fl = None
            for j in range(4):
                fl = build_instance(j, fl)
            fl()

    nc.compile()
    return nc


_prog_cache = {}


def _get_program(sks):
    if sks not in _prog_cache:
        _prog_cache[sks] = _build_program(sks)
    return _prog_cache[sks]


def kernel(q, kv, key_padding_mask):
    global LAST_EXEC_NS
    q = np.asarray(q, dtype=np.float32)
    kv = np.asarray(kv, dtype=np.float32)
    mask = np.asarray(key_padding_mask)

    sk = mask.sum(axis=1).astype(np.int64)  # (B,) valid key counts
    c = (SQ - sk).astype(np.int64)
    prog = _get_program((int(sk[0]), int(sk[1])))

    k_all = kv[:, :, 0]  # (B, SK, HK, D)
    v_all = kv[:, :, 1]

    tri = (np.arange(P)[None, :] >= np.arange(P)[:, None]).astype(np.float32)

    kT_by_g = {}
    vp_by_g = {}
    for g in range(HK):
        kTg = np.zeros((B, P, SK), dtype=np.float32)
        kTg[:, :D, :] = k_all[:, :, g, :].transpose(0, 2, 1)
        kT_by_g[g] = kTg
        vpz = np.ones((B, SK, 65), dtype=np.float32)
        vpz[:, :, :64] = v_all[:, :, g, :]
        vp = vpz.reshape(B, SK // P, P, 65).transpose(0, 2, 1, 3)
        vp_by_g[g] = np.ascontiguousarray(vp.reshape(B, P, (SK // P) * 65))

    def core_instances(core):
        g = core // 2
        hp = core % 2
        h0 = 4 * g + 2 * hp
        return g, [(0, h0), (0, h0 + 1), (1, h0), (1, h0 + 1)]

    in_maps = []
    for core in range(NCORES):
        g, insts = core_instances(core)
        qT = np.zeros((4, P, SQ), dtype=np.float32)
        for jj, (b, h) in enumerate(insts):
            U = int(sk[b])
            qT[jj, :D, :U] = q[b, c[b]:, h, :].T
        in_maps.append({
            "qT": qT.astype(BF16),
            "kT": kT_by_g[g].astype(BF16),
            "vp": vp_by_g[g].astype(BF16),
            "tri": tri.astype(BF16),
        })

    trace = bool(os.environ.get("BASS_KERNEL_TRACE"))
    res = run_bass_kernel_spmd(prog, in_maps, list(range(NCORES)),
                               trace=trace)
    LAST_EXEC_NS = res.exec_time_ns

    out = np.empty((B, SQ, H, D), dtype=np.float32)
    # fully-masked rows: uniform softmax over all SK keys -> mean of v
    vmean = v_all.mean(axis=1)  # (B, HK, D)
    for b in range(B):
        if c[b] > 0:
            for g in range(HK):
                for h in range(4 * g, 4 * g + 4):
                    out[b, :c[b], h, :] = vmean[b, g]

    for core in range(NCORES):
        g, insts = core_instances(core)
        o = res.results[core]["outT"]  # (4, 65, SQ)
        for jj, (b, h) in enumerate(insts):
            U = int(sk[b])
            num = o[jj, :64, :U]
            den = o[jj, 64, :U]
            out[b, c[b]:, h, :] = (num / den[None, :]).T

    return out



# revision 38
# speedup vs baseline: 1.0314x; 1.0290x over previous
"""GQA cross-attention kernel for Trainium2 (8 NeuronCores, Bass/Tile).

Problem: q (2,2048,16,64) f32, kv (2,2048,2,4,64) f32, key_padding_mask (2,2048)
bool.  Reference: GQA attention with additive -10000 padding bias and a causal
mask shifted by the per-batch valid key count sk, softmax over keys.

Key observations used here:
  * Every padded key position is also causal-masked (the where() sets those
    scores to exactly -10000), so only the shifted-causal structure matters.
  * With u := q_idx - c (c = 2048 - sk), the valid region is exactly u >= k,
    a standard causal triangle, and only keys k < sk participate.  The shift
    is applied on the HOST when laying out Q^T per core, so the device
    program is a static causal flash-attention kernel.
  * Rows q_idx < c have no valid key: the reference softmaxes a row of equal
    -10000s -> uniform weights -> output = mean over ALL 2048 v rows.  Pure
    host-side fixup.
  * exp without max-subtraction is safe (|score*0.125| <~ 8), and the softmax
    denominator is obtained by appending a ones-column to V (PV matmul then
    yields [num | den]); the division happens on host.

Device program (per core, 4 head-instances = 2 heads x 2 batches, mixed
batch sharding so every core gets an identical causal workload):
  S^T[k,u] = K^T.T @ Q^T   (bf16 matmuls; D zero-padded 64->128 so every
                            matmul runs K=128: avoids PE row-group switches)
  P^T      = exp(0.125 * S^T)        (ScalarE, PSUM -> SBUF, per-row tiles)
  diagonal 128x128 blocks masked by a host-provided triangle (VectorE mul)
  [num|den]^T += V'(k-tile).T @ P^T  (bf16, chunk-serial PSUM accumulation,
                            drip-scheduled 2 rows behind exp so the in-order
                            PE stream never blocks and holds 2.4 GHz)
  PSUM -> SBUF copy (VectorE), DMA out^T [65, 2048] per instance.
"""

import os
import ml_dtypes
import numpy as np

BF16 = ml_dtypes.bfloat16

import concourse.bass as bass
import concourse.mybir as mybir
import concourse.tile as tile
from concourse import bacc
from concourse.bass_utils import run_bass_kernel_spmd

B, SQ, SK, H, HK, D = 2, 2048, 2048, 16, 4, 64
NCORES = 8
P = 128
FP = mybir.dt.float32
FR = mybir.dt.bfloat16
S_TILE = 1024  # width of one PSUM scores strip (2 banks)
ACC_W = 512    # width of one PV accumulator chunk (1 bank)

LAST_EXEC_NS = None


def _ceil_div(a, b):
    return -(-a // b)


def _build_program(sks):
    """Build + compile the SPMD program for per-batch valid key counts sks.

    Schedule: per key-tile row kt, QK strips ([128, <=1536] PSUM, 3 banks,
    double-buffered) -> one exp ACT per strip into a per-row P^T tile that
    persists in SBUF -> diagonal tri-mask on DVE.  PV runs chunk-serially:
    once all rows a 512-wide output chunk needs are exp'd, a burst of PV
    matmuls accumulates that chunk in one of two rotating PSUM accumulators.
    This keeps the PE instruction stream dependency-clean (it ramps to
    2.4 GHz only when never blocked) and minimizes Scalar ACT count (the
    exp engine is the pacer at ~0.83 ns/col + ~150ns per instruction).
    """
    nc = bacc.Bacc("TRN2", target_bir_lowering=False, debug=False,
                   num_devices=NCORES)

    # D=64 zero-padded to 128 contraction rows: every matmul then runs with
    # K=128, avoiding PE row-group reconfiguration between QK (else K=64)
    # and PV (K=128) instructions, which costs ~0.2-0.4 ns/col in practice
    qT_d = nc.dram_tensor("qT", [4, P, SQ], FR, kind="ExternalInput").ap()
    kT_d = nc.dram_tensor("kT", [B, P, SK], FR, kind="ExternalInput").ap()
    vp_d = nc.dram_tensor("vp", [B, P, (SK // P) * 65], FR,
                          kind="ExternalInput").ap()
    tri_d = nc.dram_tensor("tri", [P, P], FR, kind="ExternalInput").ap()
    out_d = nc.dram_tensor("outT", [4, 65, SQ], FP, kind="ExternalOutput").ap()

    EXP = mybir.ActivationFunctionType.Exp
    S_W = 1024   # QK strip width: 2 PSUM banks
    S_BUFS = 3   # strip buffers in rotation (3 x 2 banks + 2 acc banks = 8)

    with tile.TileContext(nc) as tc:
        with (
            tc.tile_pool(name="const", bufs=1) as cpool,
            tc.tile_pool(name="kv", bufs=1) as kvpool,
            tc.tile_pool(name="qin", bufs=1) as qpool,
            tc.tile_pool(name="pt", bufs=2) as ppool,
            tc.tile_pool(name="oc", bufs=2) as opool,
            tc.tile_pool(name="ps", bufs=3, space="PSUM") as spool,
            tc.tile_pool(name="pa", bufs=2, space="PSUM") as apool,
        ):
            kT_sb = []
            vp_sb = []
            for b in range(B):
                kt_t = kvpool.tile([P, SK], FR, name=f"kT{b}", tag=f"kT{b}")
                kT_sb.append(kt_t)
                vp_t = kvpool.tile([P, (SK // P) * 65], FR, name=f"vp{b}",
                                   tag=f"vp{b}")
                vp_sb.append(vp_t)
            tri_sb = cpool.tile([P, P], FR, name="tri_sb")
            q_tiles = [qpool.tile([P, SQ], FR, name=f"q{j}", tag=f"q{j}")
                       for j in range(4)]
            # preloads ordered so the first QK matmul (needs kT0[:, :128] and
            # q0[:, :512]) unblocks after ~2 transfers; DMA issue on the sync
            # queue is serial (~0.7us each) so keep the count low
            nc.sync.dma_start(kT_sb[0][:, 0:128], kT_d[0][:, 0:128])
            nc.sync.dma_start(q_tiles[0][:, 0:512], qT_d[0][:, 0:512])
            nc.sync.dma_start(q_tiles[0][:, 512:1024], qT_d[0][:, 512:1024])
            nc.sync.dma_start(q_tiles[0][:, 1024:1536], qT_d[0][:, 1024:1536])
            nc.sync.dma_start(kT_sb[0][:, 128:512], kT_d[0][:, 128:512])
            nc.sync.dma_start(q_tiles[0][:, 1536:2048], qT_d[0][:, 1536:2048])
            nc.sync.dma_start(tri_sb[:], tri_d[:])
            nc.sync.dma_start(vp_sb[0][:], vp_d[0])
            nc.sync.dma_start(kT_sb[0][:, 512:2048], kT_d[0][:, 512:2048])
            nc.sync.dma_start(q_tiles[1][:], qT_d[1])
            nc.sync.dma_start(kT_sb[1][:], kT_d[1])
            nc.sync.dma_start(vp_sb[1][:], vp_d[1])
            nc.sync.dma_start(q_tiles[2][:], qT_d[2])
            nc.sync.dma_start(q_tiles[3][:], qT_d[3])

            alloc_state = {"n": 0}  # global strip-allocation counter

            def build_instance(j, prev_flush):
                """Emit one head-instance; returns its tail-flush closure.

                prev_flush (the previous instance's leftover PV items + final
                out-DMA) is emitted after this instance's first two QK rows:
                those items depend on the previous instance's last exps, so
                deferring them keeps the in-order PE stream unblocked at the
                instance boundary.
                """
                b = 0 if j < 2 else 1
                U = sks[b]
                KT = _ceil_div(U, P)
                NCH = _ceil_div(U, ACC_W)

                q_sb = q_tiles[j]
                pt_rows = [ppool.tile([P, SQ], FR, name=f"pt{kt}",
                                      tag=f"pt{kt}") for kt in range(KT)]
                oc = opool.tile([65, SQ], FP, name="oc", tag="oc")

                def chunk_last(c):
                    return min(KT - 1, ((c + 1) * ACC_W - 1) // P)

                # PV work list, chunk-major (accs rotate through 2 PSUM
                # buffers, so chunks must open/close in order)
                pv_items = [(c, kt2) for c in range(NCH)
                            for kt2 in range(chunk_last(c) + 1)]
                pv_state = {"i": 0, "acc": None}
                exp_last_alloc = {}  # row -> alloc index of its last strip

                def emit_pv_item(c, kt2):
                    last = chunk_last(c)
                    a0c = c * ACC_W
                    a1 = min(U, (c + 1) * ACC_W)
                    if kt2 == 0:
                        pv_state["acc"] = apool.tile([65, ACC_W], FP,
                                                     name="acc", tag="acc")
                    acc = pv_state["acc"]
                    a0 = max(P * kt2, a0c)
                    # full 128 contraction rows even on the tail key-tile:
                    # its invalid P^T rows are zeroed once, so the (real,
                    # finite) vp values there contribute exactly zero
                    nc.tensor.matmul(
                        acc[:, a0 - a0c:a1 - a0c],
                        lhsT=vp_sb[b][:, 65 * kt2:65 * (kt2 + 1)],
                        rhs=pt_rows[kt2][:, a0:a1],
                        start=(kt2 == 0), stop=(kt2 == last),
                        skip_group_check=True,
                    )
                    if kt2 == last:
                        nc.vector.tensor_copy(oc[:, a0c:a1],
                                              acc[:, 0:a1 - a0c])

                def drip_pv(thresh):
                    # emit PV items whose exp strip is at least S_BUFS strip
                    # allocations behind the current QK row's first strip:
                    # issuing that QK implies those exps completed (buffer
                    # rotation), so the items never block the in-order PE
                    # stream.  thresh=None flushes everything (tail).
                    while pv_state["i"] < len(pv_items):
                        c, kt2 = pv_items[pv_state["i"]]
                        if thresh is not None and (
                                kt2 not in exp_last_alloc
                                or exp_last_alloc[kt2] > thresh):
                            break
                        emit_pv_item(c, kt2)
                        pv_state["i"] += 1

                for kt in range(KT):
                    u0 = P * kt
                    kw = min(P, U - u0)
                    if kw < P:
                        # zero the tail key-tile's invalid P^T rows so PV can
                        # use the full 128 contraction rows; runs before exp,
                        # which then overwrites the valid rows below kw.
                        # (engine partition base must be 32-aligned)
                        st = (kw // 32) * 32
                        nc.vector.memset(pt_rows[kt][st:P, u0:U], 0.0)
                    row_first_alloc = alloc_state["n"] + 1
                    s0 = u0
                    while s0 < U:
                        # instance 0 row 0 leads with a 512-wide strip so the
                        # first exp fires after only 128KB of q has arrived
                        w_lim = 512 if (j == 0 and kt == 0 and s0 == u0) \
                            else S_W
                        s1 = min(U, s0 + w_lim)
                        alloc_state["n"] += 1
                        ps = spool.tile([P, S_W], FP, name="ps", tag="ps")
                        # <=512-wide matmul pieces: each output must stay
                        # inside one PSUM bank (bank = 512 fp32 cols)
                        m0 = s0
                        while m0 < s1:
                            m1 = min(s1, m0 + 512)
                            nc.tensor.matmul(
                                ps[0:kw, m0 - s0:m1 - s0],
                                lhsT=kT_sb[b][:, u0:u0 + kw],
                                rhs=q_sb[:, m0:m1],
                                start=True, stop=True,
                                skip_group_check=True,
                            )
                            m0 = m1
                        nc.scalar.activation(pt_rows[kt][0:kw, s0:s1],
                                             ps[0:kw, 0:s1 - s0],
                                             EXP, scale=0.125)
                        s0 = s1
                    exp_last_alloc[kt] = alloc_state["n"]
                    dw = min(P, U - u0)
                    nc.vector.tensor_mul(pt_rows[kt][0:kw, u0:u0 + dw],
                                         pt_rows[kt][0:kw, u0:u0 + dw],
                                         tri_sb[0:kw, 0:dw])
                    drip_pv(row_first_alloc - S_BUFS)
                    if kt == 1 and prev_flush is not None:
                        prev_flush()
                    if kt == KT - 1 and U > 1152:
                        # first half of the output can ship while the tail
                        # chunks are still accumulating
                        nc.sync.dma_start(out_d[j, :, 0:1024], oc[:, 0:1024])

                def flush():
                    drip_pv(None)
                    if U > 1152:
                        nc.sync.dma_start(out_d[j, :, 1024:U], oc[:, 1024:U])
                    else:
                        nc.sync.dma_start(out_d[j, :, 0:U], oc[:, 0:U])
                return flush

            fl = None
            for j in range(4):
                fl = build_instance(j, fl)
            fl()

    nc.compile()
    return nc


_prog_cache = {}


def _get_program(sks):
    if sks not in _prog_cache:
        _prog_cache[sks] = _build_program(sks)
    return _prog_cache[sks]


def kernel(q, kv, key_padding_mask):
    global LAST_EXEC_NS
    q = np.asarray(q, dtype=np.float32)
    kv = np.asarray(kv, dtype=np.float32)
    mask = np.asarray(key_padding_mask)

    sk = mask.sum(axis=1).astype(np.int64)  # (B,) valid key counts
    c = (SQ - sk).astype(np.int64)
    prog = _get_program((int(sk[0]), int(sk[1])))

    k_all = kv[:, :, 0]  # (B, SK, HK, D)
    v_all = kv[:, :, 1]

    tri = (np.arange(P)[None, :] >= np.arange(P)[:, None]).astype(np.float32)

    kT_by_g = {}
    vp_by_g = {}
    for g in range(HK):
        kTg = np.zeros((B, P, SK), dtype=np.float32)
        kTg[:, :D, :] = k_all[:, :, g, :].transpose(0, 2, 1)
        kT_by_g[g] = kTg
        vpz = np.ones((B, SK, 65), dtype=np.float32)
        vpz[:, :, :64] = v_all[:, :, g, :]
        vp = vpz.reshape(B, SK // P, P, 65).transpose(0, 2, 1, 3)
        vp_by_g[g] = np.ascontiguousarray(vp.reshape(B, P, (SK // P) * 65))

    def core_instances(core):
        g = core // 2
        hp = core % 2
        h0 = 4 * g + 2 * hp
        return g, [(0, h0), (0, h0 + 1), (1, h0), (1, h0 + 1)]

    in_maps = []
    for core in range(NCORES):
        g, insts = core_instances(core)
        qT = np.zeros((4, P, SQ), dtype=np.float32)
        for jj, (b, h) in enumerate(insts):
            U = int(sk[b])
            qT[jj, :D, :U] = q[b, c[b]:, h, :].T
        in_maps.append({
            "qT": qT.astype(BF16),
            "kT": kT_by_g[g].astype(BF16),
            "vp": vp_by_g[g].astype(BF16),
            "tri": tri.astype(BF16),
        })

    trace = bool(os.environ.get("BASS_KERNEL_TRACE"))
    res = run_bass_kernel_spmd(prog, in_maps, list(range(NCORES)),
                               trace=trace)
    LAST_EXEC_NS = res.exec_time_ns

    out = np.empty((B, SQ, H, D), dtype=np.float32)
    # fully-masked rows: uniform softmax over all SK keys -> mean of v
    vmean = v_all.mean(axis=1)  # (B, HK, D)
    for b in range(B):
        if c[b] > 0:
            for g in range(HK):
                for h in range(4 * g, 4 * g + 4):
                    out[b, :c[b], h, :] = vmean[b, g]

    for core in range(NCORES):
        g, insts = core_instances(core)
        o = res.results[core]["outT"]  # (4, 65, SQ)
        for jj, (b, h) in enumerate(insts):
            U = int(sk[b])
            num = o[jj, :64, :U]
            den = o[jj, 64, :U]
            out[b, c[b]:, h, :] = (num / den[None, :]).T

    return out



# revision 51
# speedup vs baseline: 1.0530x; 1.0209x over previous
"""GQA cross-attention kernel for Trainium2 (8 NeuronCores, Bass/Tile).

Problem: q (2,2048,16,64) f32, kv (2,2048,2,4,64) f32, key_padding_mask (2,2048)
bool.  Reference: GQA attention with additive -10000 padding bias and a causal
mask shifted by the per-batch valid key count sk, softmax over keys.

Key observations used here:
  * Every padded key position is also causal-masked (the where() sets those
    scores to exactly -10000), so only the shifted-causal structure matters.
  * With u := q_idx - c (c = 2048 - sk), the valid region is exactly u >= k,
    a standard causal triangle, and only keys k < sk participate.  The shift
    is applied on the HOST when laying out Q^T per core, so the device
    program is a static causal flash-attention kernel.
  * Rows q_idx < c have no valid key: the reference softmaxes a row of equal
    -10000s -> uniform weights -> output = mean over ALL 2048 v rows.  Pure
    host-side fixup.
  * exp without max-subtraction is safe (|score*0.125| <~ 8), and the softmax
    denominator is obtained by appending a ones-column to V (PV matmul then
    yields [num | den]); the division happens on host.

Device program (per core, 4 head-instances = 2 heads x 2 batches, mixed
batch sharding so every core gets an identical causal workload):
  S^T[k,u] = K^T.T @ Q^T   (bf16 matmuls; D zero-padded 64->128 so every
                            matmul runs K=128: avoids PE row-group switches)
  P^T      = exp(0.125 * S^T)        (ScalarE, PSUM -> SBUF, per-row tiles)
  diagonal 128x128 blocks masked by a host-provided triangle (VectorE mul)
  [num|den]^T += V'(k-tile).T @ P^T  (bf16, chunk-serial PSUM accumulation,
                            drip-scheduled 2 rows behind exp so the in-order
                            PE stream never blocks and holds 2.4 GHz)
  PSUM -> SBUF copy (VectorE), DMA out^T [65, 2048] per instance.
"""

import os
import ml_dtypes
import numpy as np

BF16 = ml_dtypes.bfloat16

import concourse.bass as bass
import concourse.mybir as mybir
import concourse.tile as tile
from concourse import bacc
from concourse.bass_utils import run_bass_kernel_spmd

B, SQ, SK, H, HK, D = 2, 2048, 2048, 16, 4, 64
NCORES = 8
P = 128
FP = mybir.dt.float32
FR = mybir.dt.bfloat16
S_TILE = 1024  # width of one PSUM scores strip (2 banks)
ACC_W = 512    # width of one PV accumulator chunk (1 bank)

LAST_EXEC_NS = None


def _ceil_div(a, b):
    return -(-a // b)


def _build_program(sks):
    """Build + compile the SPMD program for per-batch valid key counts sks.

    Schedule: per key-tile row kt, QK strips ([128, <=1536] PSUM, 3 banks,
    double-buffered) -> one exp ACT per strip into a per-row P^T tile that
    persists in SBUF -> diagonal tri-mask on DVE.  PV runs chunk-serially:
    once all rows a 512-wide output chunk needs are exp'd, a burst of PV
    matmuls accumulates that chunk in one of two rotating PSUM accumulators.
    This keeps the PE instruction stream dependency-clean (it ramps to
    2.4 GHz only when never blocked) and minimizes Scalar ACT count (the
    exp engine is the pacer at ~0.83 ns/col + ~150ns per instruction).
    """
    nc = bacc.Bacc("TRN2", target_bir_lowering=False, debug=False,
                   num_devices=NCORES)

    # q/kT SBUF tiles have 128 contraction rows; rows 64:128 are zeroed
    # on-device (DVE memsets, proven at partition base 64) so every matmul
    # runs K=128 -- avoiding PE row-group reconfiguration between QK (else
    # K=64) and PV (K=128) -- while the dram tensors stay D=64, halving the
    # preload DMA volume vs host-side padding
    qT_d = nc.dram_tensor("qT", [4, D, SQ], FR, kind="ExternalInput").ap()
    kT_d = nc.dram_tensor("kT", [B, D, SK], FR, kind="ExternalInput").ap()
    vp_d = nc.dram_tensor("vp", [B, P, (SK // P) * 65], FR,
                          kind="ExternalInput").ap()
    tri_d = nc.dram_tensor("tri", [P, P], FR, kind="ExternalInput").ap()
    out_d = nc.dram_tensor("outT", [4, 65, SQ], FP, kind="ExternalOutput").ap()

    EXP = mybir.ActivationFunctionType.Exp
    S_W = 1024   # QK strip width: 2 PSUM banks
    S_BUFS = 3   # strip buffers in rotation (3 x 2 banks + 2 acc banks = 8)

    with tile.TileContext(nc) as tc:
        with (
            tc.tile_pool(name="const", bufs=1) as cpool,
            tc.tile_pool(name="kv", bufs=1) as kvpool,
            tc.tile_pool(name="qin", bufs=1) as qpool,
            tc.tile_pool(name="pt", bufs=2) as ppool,
            tc.tile_pool(name="oc", bufs=2) as opool,
            tc.tile_pool(name="ps", bufs=3, space="PSUM") as spool,
            tc.tile_pool(name="pa", bufs=2, space="PSUM") as apool,
        ):
            kT_sb = []
            vp_sb = []
            for b in range(B):
                kt_t = kvpool.tile([P, SK], FR, name=f"kT{b}", tag=f"kT{b}")
                kT_sb.append(kt_t)
                vp_t = kvpool.tile([P, (SK // P) * 65], FR, name=f"vp{b}",
                                   tag=f"vp{b}")
                vp_sb.append(vp_t)
            tri_sb = cpool.tile([P, P], FR, name="tri_sb")
            q_tiles = [qpool.tile([P, SQ], FR, name=f"q{j}", tag=f"q{j}")
                       for j in range(4)]
            # preloads ordered so the first QK matmul (needs kT0[:, :128] and
            # q0[:, :512]) unblocks after ~2 transfers; DMA issue on the sync
            # queue is serial (~0.7us each) so keep the count low
            # pad-row zeroing for the tiles needed first (rest emitted inside
            # the instance flow so they don't block the DVE mask stream)
            nc.vector.memset(kT_sb[0][D:P, 0:512], 0.0)
            nc.vector.memset(q_tiles[0][D:P, 0:1024], 0.0)
            nc.sync.dma_start(kT_sb[0][0:D, 0:128], kT_d[0][:, 0:128])
            nc.sync.dma_start(q_tiles[0][0:D, 0:512], qT_d[0][:, 0:512])
            nc.sync.dma_start(q_tiles[0][0:D, 512:1024], qT_d[0][:, 512:1024])
            nc.sync.dma_start(q_tiles[0][0:D, 1024:1536],
                              qT_d[0][:, 1024:1536])
            nc.sync.dma_start(kT_sb[0][0:D, 128:512], kT_d[0][:, 128:512])
            nc.sync.dma_start(q_tiles[0][0:D, 1536:2048],
                              qT_d[0][:, 1536:2048])
            nc.sync.dma_start(tri_sb[:], tri_d[:])
            nc.sync.dma_start(vp_sb[0][:], vp_d[0])
            nc.sync.dma_start(kT_sb[0][0:D, 512:2048], kT_d[0][:, 512:2048])
            nc.sync.dma_start(q_tiles[1][0:D, :], qT_d[1])
            nc.sync.dma_start(kT_sb[1][0:D, :], kT_d[1])
            nc.sync.dma_start(vp_sb[1][:], vp_d[1])
            nc.sync.dma_start(q_tiles[2][0:D, :], qT_d[2])
            nc.sync.dma_start(q_tiles[3][0:D, :], qT_d[3])
            nc.vector.memset(kT_sb[0][D:P, 512:2048], 0.0)
            nc.vector.memset(q_tiles[0][D:P, 1024:2048], 0.0)

            alloc_state = {"n": 0}  # global strip-allocation counter

            def build_instance(j, prev_flush):
                """Emit one head-instance; returns its tail-flush closure.

                prev_flush (the previous instance's leftover PV items + final
                out-DMA) is emitted after this instance's first two QK rows:
                those items depend on the previous instance's last exps, so
                deferring them keeps the in-order PE stream unblocked at the
                instance boundary.
                """
                b = 0 if j < 2 else 1
                U = sks[b]
                KT = _ceil_div(U, P)
                NCH = _ceil_div(U, ACC_W)

                q_sb = q_tiles[j]
                pt_rows = [ppool.tile([P, SQ], FR, name=f"pt{kt}",
                                      tag=f"pt{kt}") for kt in range(KT)]
                oc = opool.tile([65, SQ], FP, name="oc", tag="oc")

                def chunk_last(c):
                    return min(KT - 1, ((c + 1) * ACC_W - 1) // P)

                # PV work list, chunk-major (accs rotate through 2 PSUM
                # buffers, so chunks must open/close in order)
                pv_items = [(c, kt2) for c in range(NCH)
                            for kt2 in range(chunk_last(c) + 1)]
                pv_state = {"i": 0, "acc": None}
                exp_last_alloc = {}  # row -> alloc index of its last strip

                def emit_pv_item(c, kt2):
                    last = chunk_last(c)
                    a0c = c * ACC_W
                    a1 = min(U, (c + 1) * ACC_W)
                    if kt2 == 0:
                        pv_state["acc"] = apool.tile([65, ACC_W], FP,
                                                     name="acc", tag="acc")
                    acc = pv_state["acc"]
                    a0 = max(P * kt2, a0c)
                    # full 128 contraction rows even on the tail key-tile:
                    # its invalid P^T rows are zeroed once, so the (real,
                    # finite) vp values there contribute exactly zero
                    nc.tensor.matmul(
                        acc[:, a0 - a0c:a1 - a0c],
                        lhsT=vp_sb[b][:, 65 * kt2:65 * (kt2 + 1)],
                        rhs=pt_rows[kt2][:, a0:a1],
                        start=(kt2 == 0), stop=(kt2 == last),
                        skip_group_check=True,
                    )
                    if kt2 == last:
                        nc.vector.tensor_copy(oc[:, a0c:a1],
                                              acc[:, 0:a1 - a0c])

                def drip_pv(thresh):
                    # emit PV items whose exp strip is at least S_BUFS strip
                    # allocations behind the current QK row's first strip:
                    # issuing that QK implies those exps completed (buffer
                    # rotation), so the items never block the in-order PE
                    # stream.  thresh=None flushes everything (tail).
                    while pv_state["i"] < len(pv_items):
                        c, kt2 = pv_items[pv_state["i"]]
                        if thresh is not None and (
                                kt2 not in exp_last_alloc
                                or exp_last_alloc[kt2] > thresh):
                            break
                        emit_pv_item(c, kt2)
                        pv_state["i"] += 1

                for kt in range(KT):
                    u0 = P * kt
                    kw = min(P, U - u0)
                    if kw < P:
                        # zero the tail key-tile's invalid P^T rows so PV can
                        # use the full 128 contraction rows; runs before exp,
                        # which then overwrites the valid rows below kw.
                        # (engine partition base must be 32-aligned)
                        st = (kw // 32) * 32
                        nc.vector.memset(pt_rows[kt][st:P, u0:U], 0.0)
                    row_first_alloc = alloc_state["n"] + 1
                    s0 = u0
                    while s0 < U:
                        # instance 0 row 0 leads with a 512-wide strip so the
                        # first exp fires after only 128KB of q has arrived
                        w_lim = 512 if (j == 0 and kt == 0 and s0 == u0) \
                            else S_W
                        s1 = min(U, s0 + w_lim)
                        alloc_state["n"] += 1
                        ps = spool.tile([P, S_W], FP, name="ps", tag="ps")
                        # <=512-wide matmul pieces: each output must stay
                        # inside one PSUM bank (bank = 512 fp32 cols)
                        m0 = s0
                        while m0 < s1:
                            m1 = min(s1, m0 + 512)
                            nc.tensor.matmul(
                                ps[0:kw, m0 - s0:m1 - s0],
                                lhsT=kT_sb[b][:, u0:u0 + kw],
                                rhs=q_sb[:, m0:m1],
                                start=True, stop=True,
                                skip_group_check=True,
                            )
                            m0 = m1
                        nc.scalar.activation(pt_rows[kt][0:kw, s0:s1],
                                             ps[0:kw, 0:s1 - s0],
                                             EXP, scale=0.125)
                        s0 = s1
                    exp_last_alloc[kt] = alloc_state["n"]
                    dw = min(P, U - u0)
                    nc.vector.tensor_mul(pt_rows[kt][0:kw, u0:u0 + dw],
                                         pt_rows[kt][0:kw, u0:u0 + dw],
                                         tri_sb[0:kw, 0:dw])
                    drip_pv(row_first_alloc - S_BUFS)
                    if kt == 2:
                        # pad-row zeroing for upcoming instances' tiles,
                        # spread through the DVE stream well ahead of use
                        if j == 0:
                            nc.vector.memset(q_tiles[1][D:P, :], 0.0)
                        elif j == 1:
                            nc.vector.memset(kT_sb[1][D:P, :], 0.0)
                            nc.vector.memset(q_tiles[2][D:P, :], 0.0)
                        elif j == 2:
                            nc.vector.memset(q_tiles[3][D:P, :], 0.0)
                    if kt == 1 and prev_flush is not None:
                        prev_flush()
                    if kt == KT - 1 and U > 1152:
                        # first half of the output can ship while the tail
                        # chunks are still accumulating
                        nc.sync.dma_start(out_d[j, :, 0:1024], oc[:, 0:1024])

                def flush():
                    drip_pv(None)
                    if U > 1152:
                        nc.sync.dma_start(out_d[j, :, 1024:U], oc[:, 1024:U])
                    else:
                        nc.sync.dma_start(out_d[j, :, 0:U], oc[:, 0:U])
                return flush

            fl = None
            for j in range(4):
                fl = build_instance(j, fl)
            fl()

    nc.compile()
    return nc


_prog_cache = {}


def _get_program(sks):
    if sks not in _prog_cache:
        _prog_cache[sks] = _build_program(sks)
    return _prog_cache[sks]


def kernel(q, kv, key_padding_mask):
    global LAST_EXEC_NS
    q = np.asarray(q, dtype=np.float32)
    kv = np.asarray(kv, dtype=np.float32)
    mask = np.asarray(key_padding_mask)

    sk = mask.sum(axis=1).astype(np.int64)  # (B,) valid key counts
    c = (SQ - sk).astype(np.int64)
    prog = _get_program((int(sk[0]), int(sk[1])))

    k_all = kv[:, :, 0]  # (B, SK, HK, D)
    v_all = kv[:, :, 1]

    tri = (np.arange(P)[None, :] >= np.arange(P)[:, None]).astype(np.float32)

    kT_by_g = {}
    vp_by_g = {}
    for g in range(HK):
        kT_by_g[g] = np.ascontiguousarray(
            k_all[:, :, g, :].transpose(0, 2, 1))  # (B, D, SK)
        vpz = np.ones((B, SK, 65), dtype=np.float32)
        vpz[:, :, :64] = v_all[:, :, g, :]
        vp = vpz.reshape(B, SK // P, P, 65).transpose(0, 2, 1, 3)
        vp_by_g[g] = np.ascontiguousarray(vp.reshape(B, P, (SK // P) * 65))

    def core_instances(core):
        g = core // 2
        hp = core % 2
        h0 = 4 * g + 2 * hp
        return g, [(0, h0), (0, h0 + 1), (1, h0), (1, h0 + 1)]

    in_maps = []
    for core in range(NCORES):
        g, insts = core_instances(core)
        qT = np.zeros((4, D, SQ), dtype=np.float32)
        for jj, (b, h) in enumerate(insts):
            U = int(sk[b])
            qT[jj, :, :U] = q[b, c[b]:, h, :].T
        in_maps.append({
            "qT": qT.astype(BF16),
            "kT": kT_by_g[g].astype(BF16),
            "vp": vp_by_g[g].astype(BF16),
            "tri": tri.astype(BF16),
        })

    trace = bool(os.environ.get("BASS_KERNEL_TRACE"))
    res = run_bass_kernel_spmd(prog, in_maps, list(range(NCORES)),
                               trace=trace)
    LAST_EXEC_NS = res.exec_time_ns

    out = np.empty((B, SQ, H, D), dtype=np.float32)
    # fully-masked rows: uniform softmax over all SK keys -> mean of v
    vmean = v_all.mean(axis=1)  # (B, HK, D)
    for b in range(B):
        if c[b] > 0:
            for g in range(HK):
                for h in range(4 * g, 4 * g + 4):
                    out[b, :c[b], h, :] = vmean[b, g]

    for core in range(NCORES):
        g, insts = core_instances(core)
        o = res.results[core]["outT"]  # (4, 65, SQ)
        for jj, (b, h) in enumerate(insts):
            U = int(sk[b])
            num = o[jj, :64, :U]
            den = o[jj, 64, :U]
            out[b, c[b]:, h, :] = (num / den[None, :]).T

    return out

